# revision 33
# baseline (speedup 1.0000x reference)
# Trainium2 Bass kernel for nn_AttentionPropagation (SuperGlue-style bidirectional
# attentional propagation): 6x (1x1conv+BN+ReLU) filters + QK attention with
# softmax over BOTH axes + two aggregations + output filters.
#
# Sharding: 16 (batch, head) units over 8 cores -> core owns batch b=core//2
# and a 128-channel (2-head) slice. Pair {2b, 2b+1} AllGathers add0/add1.
#
# v2: single exp stream. E = exp(QK/8) is computed ONCE (ACT-bound ~73us);
# the transposed copies F = E^T needed for the U0 aggregation are produced by
# DMA xbar transposes (idle DMA engines) instead of a second QK^T+exp pass.
# Colsums come from concurrent col-tiled ones-matmuls during the stream;
# rowsums from exp accum_out. Normalizer reorder chains use xbar transpose +
# small DRAM roundtrips, overlapped with the stream. AllGather split per
# tensor so the out1 path starts while U0 is still accumulating.

import numpy as np

B, N, M, C = 4, 2048, 2048, 256
H, Dh = 4, 64
EPS = 1e-5
NCORES = 8

_CACHE = {}


def _build_program():
    from contextlib import ExitStack

    import concourse.bass as bass
    import concourse.tile as tile
    from concourse import bacc, mybir
    from concourse.bass import ts

    f32 = mybir.dt.float32
    bf16 = mybir.dt.bfloat16
    AF = mybir.ActivationFunctionType
    ALU = mybir.AluOpType

    nc = bacc.Bacc(
        "TRN2",
        target_bir_lowering=False,
        debug=False,
        enable_asserts=False,
        num_devices=NCORES,
    )

    # ---- DRAM I/O ----
    x1t_d = nc.dram_tensor("x1t", [C, N], bf16, kind="ExternalInput").ap()
    x2t_d = nc.dram_tensor("x2t", [C, M], bf16, kind="ExternalInput").ap()
    wq_d = nc.dram_tensor("wq", [C, 128], bf16, kind="ExternalInput").ap()
    wk_d = nc.dram_tensor("wk", [C, 128], bf16, kind="ExternalInput").ap()
    wv0_d = nc.dram_tensor("wv0", [C, 128], bf16, kind="ExternalInput").ap()
    wv1_d = nc.dram_tensor("wv1", [C, 128], bf16, kind="ExternalInput").ap()
    bq_d = nc.dram_tensor("bq", [128, 1], f32, kind="ExternalInput").ap()
    bk_d = nc.dram_tensor("bk", [128, 1], f32, kind="ExternalInput").ap()
    bv0_d = nc.dram_tensor("bv0", [1, 128], bf16, kind="ExternalInput").ap()
    bv1_d = nc.dram_tensor("bv1", [1, 128], bf16, kind="ExternalInput").ap()
    w4t_d = nc.dram_tensor("w4t", [C, C], bf16, kind="ExternalInput").ap()
    w5t_d = nc.dram_tensor("w5t", [C, C], bf16, kind="ExternalInput").ap()
    b4_d = nc.dram_tensor("b4", [1, C], bf16, kind="ExternalInput").ap()
    b5_d = nc.dram_tensor("b5", [1, C], bf16, kind="ExternalInput").ap()
    ones_d = nc.dram_tensor("ones", [1, 128], bf16, kind="ExternalInput").ap()
    onescol_d = nc.dram_tensor("onescol", [128, 1], bf16, kind="ExternalInput").ap()
    masks_d = nc.dram_tensor("masks", [2, 128], bf16, kind="ExternalInput").ap()
    out0_d = nc.dram_tensor("out0t", [N, C], f32, kind="ExternalOutput").ap()
    out1_d = nc.dram_tensor("out1t", [M, C], f32, kind="ExternalOutput").ap()
    cc1_in = nc.dram_tensor("cc1_in", [128, M], bf16, kind="Internal").ap()
    cc1_out = nc.dram_tensor("cc1_out", [256, M], bf16, kind="Internal").ap()
    cc0_in = nc.dram_tensor("cc0_in", [128, N], bf16, kind="Internal").ap()
    cc0_out = nc.dram_tensor("cc0_out", [256, N], bf16, kind="Internal").ap()
    rsc_d = nc.dram_tensor("rsc", [2, 128, 16], bf16, kind="Internal").ap()
    crec_d = nc.dram_tensor("crec", [2, M], bf16, kind="Internal").ap()
    cflat_d = nc.dram_tensor("cflat", [2, M], bf16, kind="Internal").ap()
    # E staging: one tensor per (h2, u) so Tile's whole-tensor DRAM dep
    # tracking never serializes one half's writes against the other's reads
    e4 = [
        [
            nc.dram_tensor(f"escr{h2}{u}", [N, 1024], bf16, kind="Internal").ap()
            for u in range(2)
        ]
        for h2 in range(2)
    ]

    NB = N // 128  # 16
    GRP = [[0, 1], [2, 3], [4, 5], [6, 7]]

    with tile.TileContext(nc) as tc, ExitStack() as ctx:
        const = ctx.enter_context(tc.tile_pool(name="const", bufs=1))
        # x1t/x2t (phase 1) and af0/af1 (phase 5) share two slots via one tag
        xpool = ctx.enter_context(tc.tile_pool(name="xp", bufs=2))
        qkp = ctx.enter_context(tc.tile_pool(name="qkp", bufs=1))
        vp = ctx.enter_context(tc.tile_pool(name="vp", bufs=1))
        accp = ctx.enter_context(tc.tile_pool(name="accp", bufs=1))
        flp = ctx.enter_context(tc.tile_pool(name="flp", bufs=2))
        addp = ctx.enter_context(tc.tile_pool(name="addp", bufs=1))
        bcp = ctx.enter_context(tc.tile_pool(name="bcp", bufs=1))
        stream = ctx.enter_context(tc.tile_pool(name="stream", bufs=5))
        fp = ctx.enter_context(tc.tile_pool(name="fp", bufs=2))
        opool = ctx.enter_context(tc.tile_pool(name="opool", bufs=2))
        # PSUM: psS = 2x[128,1024] (4 banks), psU = 1x[128,1024] (2 banks),
        # psC = 1x[128,1024] (2 banks) -> 8 banks exactly.
        psS = ctx.enter_context(tc.tile_pool(name="psS", bufs=2, space="PSUM"))
        psU = ctx.enter_context(tc.tile_pool(name="psU", bufs=1, space="PSUM"))
        psC = ctx.enter_context(tc.tile_pool(name="psC", bufs=1, space="PSUM"))

        # ---- constants ----
        wq_sb = const.tile([128, 2, 128], bf16, tag="wq")
        wk_sb = const.tile([128, 2, 128], bf16, tag="wk")
        wv0_sb = const.tile([128, 2, 128], bf16, tag="wv0")
        wv1_sb = const.tile([128, 2, 128], bf16, tag="wv1")
        w4t_sb = const.tile([128, 2, 256], bf16, tag="w4t")
        w5t_sb = const.tile([128, 2, 256], bf16, tag="w5t")
        for dst, src in (
            (wq_sb, wq_d), (wk_sb, wk_d), (wv0_sb, wv0_d), (wv1_sb, wv1_d),
            (w4t_sb, w4t_d), (w5t_sb, w5t_d),
        ):
            # SWDGE: keep the sync queue free for the x loads
            nc.gpsimd.dma_start(dst[:], src.rearrange("(a p) d -> p a d", p=128))
        bq_sb = const.tile([128, 1], f32, tag="bq")
        bk_sb = const.tile([128, 1], f32, tag="bk")
        bv0_sb = const.tile([1, 128], bf16, tag="bv0")
        bv1_sb = const.tile([1, 128], bf16, tag="bv1")
        b4_sb = const.tile([1, 256], bf16, tag="b4")
        b5_sb = const.tile([1, 256], bf16, tag="b5")
        ones_t = const.tile([1, 128], bf16, tag="ones")
        onescol = const.tile([128, 1], bf16, tag="onescol")
        masks = const.tile([2, 128], bf16, tag="masks")
        for dst, src in (
            (bq_sb, bq_d), (bk_sb, bk_d), (bv0_sb, bv0_d), (bv1_sb, bv1_d),
            (b4_sb, b4_d), (b5_sb, b5_d), (ones_t, ones_d), (onescol, onescol_d),
            (masks, masks_d),
        ):
            nc.sync.dma_start(dst[:], src)

        x1t_sb = xpool.tile([128, 2, N], bf16, tag="xa")
        x2t_sb = xpool.tile([128, 2, M], bf16, tag="xa")
        for a in range(2):
            nc.sync.dma_start(x1t_sb[:, a, :], x1t_d[a * 128 : (a + 1) * 128, :])
            nc.sync.dma_start(x2t_sb[:, a, :], x2t_d[a * 128 : (a + 1) * 128, :])

        # ---- phase 1: filters ----
        q_sb = qkp.tile([128, N], bf16, tag="q")
        k_sb = qkp.tile([128, M], bf16, tag="k")
        for dst, xt, w, bias in ((q_sb, x1t_sb, wq_sb, bq_sb), (k_sb, x2t_sb, wk_sb, bk_sb)):
            for j in range(4):  # 512-wide chunks
                ps = psS.tile([128, 1024], f32, tag="s")
                p5 = ps[:, 0:512]
                nc.tensor.matmul(p5, w[:, 0], xt[:, 0, ts(j, 512)], start=True, stop=False)
                nc.tensor.matmul(p5, w[:, 1], xt[:, 1, ts(j, 512)], start=False, stop=True)
                nc.vector.tensor_scalar(
                    dst[:, ts(j, 512)], p5, bias[:], 0.0, op0=ALU.add, op1=ALU.max
                )

        v0t_sb = vp.tile([128, NB * 128], bf16, tag="v0t")  # [m-in-block, mb*128+d]
        v1t_sb = vp.tile([128, NB * 128], bf16, tag="v1t")
        for dst, xt, w, brow in (
            (v0t_sb, x2t_sb, wv0_sb, bv0_sb),
            (v1t_sb, x1t_sb, wv1_sb, bv1_sb),
        ):
            for mb in range(16):
                ps = psS.tile([128, 1024], f32, tag="s")
                p1 = ps[:, 0:128]
                nc.tensor.matmul(p1, xt[:, 0, ts(mb, 128)], w[:, 0], start=True, stop=False)
                nc.tensor.matmul(p1, xt[:, 1, ts(mb, 128)], w[:, 1], start=False, stop=False)
                nc.tensor.matmul(p1, ones_t[:, 0:128], brow[:], start=False, stop=True)
                nc.vector.tensor_scalar_max(dst[:, ts(mb, 128)], p1, 0.0)

        # rowsum accum: col index = h2*16 + nb (n on partitions)
        rows_acc = [
            accp.tile([128, 32], f32, tag=f"ra{u}", name=f"rows_acc{u}") for u in range(2)
        ]
        # colsum flat staging: u0 at partition row 64, u1 at row 0 (m on free)
        csf = accp.tile([128, M], bf16, tag="csf", name="csf")
        add0_sb = addp.tile([128, N], bf16, tag="a0")
        add1_sb = addp.tile([128, M], bf16, tag="a1")
        rbc = bcp.tile([128, N], bf16, tag="rbc")
        cbc = bcp.tile([128, M], bf16, tag="cbc")
        # F = E^T per head u: [128 m-part, mb, n]
        F_sb = [fp.tile([128, 16, N], bf16, tag="F", name=f"F{u}") for u in range(2)]

        # ---- phase 2: the single E-stream ----
        # per (h2, nb, u): QK -> exp(+rowsum) -> U1 accum + colsum MMs (col-
        # tiled, concurrent) + 8 xbar transposes into F.
        for h2 in range(2):
            psU_h = psU.tile([128, 1024], f32, tag="u", name=f"psU_h{h2}")
            cs_ps = psC.tile([128, 1024], f32, tag="c", name=f"cs_h{h2}")
            for nb in range(NB):
                for u in range(2):
                    qs = q_sb[64 * u : 64 * u + 64, ts(nb, 128)]
                    ps = psS.tile([128, 1024], f32, tag="s")
                    for j in range(2):
                        nc.tensor.matmul(
                            ps[:, ts(j, 512)], qs,
                            k_sb[64 * u : 64 * u + 64,
                                 h2 * 1024 + 512 * j : h2 * 1024 + 512 * (j + 1)],
                            start=True, stop=True,
                        )
                    et = stream.tile([128, 1024], bf16, tag="st")
                    nc.scalar.activation(
                        et[:], ps[:], AF.Exp, scale=0.125,
                        accum_out=rows_acc[u][:, h2 * 16 + nb : h2 * 16 + nb + 1],
                    )
                    # U1 accumulation (contract n): rows 64u..64u+63.
                    # start=True marks the bank's pending-zero only on the
                    # partitions this matmul writes, so each u-region opens
                    # its own accumulation group at its first nb.
                    for j in range(2):
                        nc.tensor.matmul(
                            psU_h[64 * u : 64 * u + 64, ts(j, 512)],
                            v1t_sb[:, nb * 128 + 64 * u : nb * 128 + 64 * u + 64],
                            et[:, ts(j, 512)],
                            start=(nb == 0), stop=(nb == NB - 1),
                            skip_group_check=True,
                        )
                    # colsum row (ones-matmul): u0 -> psum row 64, u1 -> row 0
                    # (different col group than the U1 matmul -> can overlap)
                    r = 64 * (1 - u)
                    for j in range(2):
                        nc.tensor.matmul(
                            cs_ps[r : r + 1, ts(j, 512)],
                            onescol[:], et[:, ts(j, 512)],
                            start=(nb == 0), stop=(nb == NB - 1),
                            skip_group_check=True,
                        )
                    # stage E to DRAM via SWDGE (async descriptor path — the
                    # HWDGE ucode DMAs occupy their queue for the whole
                    # transfer and would stall the stream)
                    nc.gpsimd.dma_start(
                        e4[h2][u][nb * 128 : (nb + 1) * 128, :], et[:]
                    )
            # stripe transposes for this half: E[:, mb-slice] -> F[u][:, mb, :]
            # (one xbar instruction per (u, mb): DRAM [2048, 128] -> [128, 2048];
            # DRAM-source xbar is the HW-validated form; SBUF-source is broken).
            # (all on SP: a scalar-queue split coincided with F corruption in
            # an earlier run — keep one queue until separately validated)
            for u in range(2):
                for j in range(8):
                    mb = h2 * 8 + j
                    nc.sync.dma_start_transpose(
                        F_sb[u][:, mb, :],
                        e4[h2][u][:, j * 128 : (j + 1) * 128],
                    )
            # ---- end of h2 half: evictions ----
            nc.vector.tensor_copy(add1_sb[:, h2 * 1024 : h2 * 1024 + 1024], psU_h[:])
            for u in range(2):
                r = 64 * (1 - u)
                nc.vector.tensor_copy(
                    csf[r : r + 1, h2 * 1024 : h2 * 1024 + 1024], cs_ps[r : r + 1, :]
                )
                # colsum flat -> DRAM (m-order: flat[m], m = p*16+i after reload)
                nc.gpsimd.dma_start(
                    cflat_d[u, h2 * 1024 : h2 * 1024 + 1024],
                    csf[r : r + 1, h2 * 1024 : h2 * 1024 + 1024],
                )

        # ---- normalizer chains ----
        # Flats land on partitions 0/1 of a [2, 2048] tile; the [128, 2048]
        # broadcast tensors are built by a K=2 mask-matmul (masks.T @ flats)
        # — sub-range partition_broadcast and SBUF-source xbar transposes are
        # broken on HW, so neither is used.
        # Col chain FIRST: it feeds add1 -> AG1 -> f5, the longest tail pole.
        # cflat[u] (m-order) -> [128,16] (m = p*16+i) -> recip -> write back
        # in m-order -> reload as a flat row.
        cfl2 = flp.tile([2, M], bf16, tag="fl", name="cfl2")
        for u in range(2):
            c16 = accp.tile([128, 16], bf16, tag=f"c16_{u}", name=f"c16_{u}")
            nc.gpsimd.dma_start(c16[:], cflat_d[u].rearrange("(p i) -> p i", i=16))
            c16f = accp.tile([128, 16], f32, tag=f"c16f_{u}", name=f"c16f_{u}")
            nc.vector.tensor_copy(c16f[:], c16[:])
            cr16 = accp.tile([128, 16], f32, tag=f"cr16_{u}", name=f"cr16_{u}")
            nc.vector.reciprocal(cr16[:], c16f[:])
            cr16b = accp.tile([128, 16], bf16, tag=f"cr16b_{u}", name=f"cr16b_{u}")
            nc.vector.tensor_copy(cr16b[:], cr16[:])
            nc.gpsimd.dma_start(crec_d[u].rearrange("(p i) -> p i", i=16), cr16b[:])
            nc.gpsimd.dma_start(cfl2[u : u + 1, :], crec_d[u : u + 1, :])
        for half in range(2):
            ps = psS.tile([128, 1024], f32, tag="s")
            for j in range(2):
                nc.tensor.matmul(
                    ps[:, ts(j, 512)], masks[:],
                    cfl2[:, half * 1024 + j * 512 : half * 1024 + (j + 1) * 512],
                    start=True, stop=True,
                )
            nc.vector.tensor_copy(cbc[:, half * 1024 : half * 1024 + 1024], ps[:])
        # add1 = U1 * cbc (in place), ship + gather immediately
        nc.vector.tensor_mul(add1_sb[:], add1_sb[:], cbc[:])
        nc.gpsimd.dma_start(cc1_in[:], add1_sb[:])
        nc.gpsimd.collective_compute(
            "AllGather", ALU.bypass, replica_groups=GRP, ins=[cc1_in], outs=[cc1_out]
        )

        # row chain: s16 -> recip -> bf16 [128,16] -> DRAM -> gather-read the
        # flat in n = i*128+p order (baseline-validated descriptor gather).
        rfl2 = flp.tile([2, N], bf16, tag="fl", name="rfl2")
        for u in range(2):
            s16 = accp.tile([128, 16], f32, tag=f"s16_{u}", name=f"s16_{u}")
            nc.vector.tensor_add(s16[:], rows_acc[u][:, 0:16], rows_acc[u][:, 16:32])
            r16 = accp.tile([128, 16], f32, tag=f"r16_{u}", name=f"r16_{u}")
            nc.vector.reciprocal(r16[:], s16[:])
            r16b = accp.tile([128, 16], bf16, tag=f"r16b_{u}", name=f"r16b_{u}")
            nc.vector.tensor_copy(r16b[:], r16[:])
            nc.gpsimd.dma_start(rsc_d[u], r16b[:])
            nc.gpsimd.dma_start(rfl2[u : u + 1, :], rsc_d[u].rearrange("p i -> i p"))
        for half in range(2):
            ps = psS.tile([128, 1024], f32, tag="s")
            for j in range(2):
                nc.tensor.matmul(
                    ps[:, ts(j, 512)], masks[:],
                    rfl2[:, half * 1024 + j * 512 : half * 1024 + (j + 1) * 512],
                    start=True, stop=True,
                )
            nc.vector.tensor_copy(rbc[:, half * 1024 : half * 1024 + 1024], ps[:])

        # ---- U0 from F (contract m), in two n-halves; normalize on eviction ----
        for nh in range(2):
            U0ps = psU.tile([128, 1024], f32, tag="u", name=f"U0_{nh}")
            for mb in range(16):
                for u in range(2):
                    for jc in range(2):
                        nc.tensor.matmul(
                            U0ps[64 * u : 64 * u + 64, ts(jc, 512)],
                            v0t_sb[:, mb * 128 + 64 * u : mb * 128 + 64 * u + 64],
                            F_sb[u][:, mb, nh * 1024 + jc * 512 : nh * 1024 + (jc + 1) * 512],
                            start=(mb == 0), stop=(mb == 15),
                            skip_group_check=True,
                        )
            nc.vector.tensor_mul(
                add0_sb[:, nh * 1024 : nh * 1024 + 1024],
                U0ps[:], rbc[:, nh * 1024 : nh * 1024 + 1024],
            )
        nc.gpsimd.dma_start(cc0_in[:], add0_sb[:])
        nc.gpsimd.collective_compute(
            "AllGather", ALU.bypass, replica_groups=GRP, ins=[cc0_in], outs=[cc0_out]
        )

        # af loads on the ACT HWDGE ring (idle post-stream) so they are not
        # stuck behind AG0 in the gpsimd FIFO
        af1 = xpool.tile([128, 2, M], bf16, tag="xa")
        nc.scalar.dma_start(af1[:, 0, :], cc1_out[0:128, :])
        nc.scalar.dma_start(af1[:, 1, :], cc1_out[128:256, :])
        af0 = xpool.tile([128, 2, N], bf16, tag="xa")
        nc.scalar.dma_start(af0[:, 0, :], cc0_out[0:128, :])
        nc.scalar.dma_start(af0[:, 1, :], cc0_out[128:256, :])

        # ---- phase 5: output filters (f5 first: af1 is ready earlier) ----
        for out_d, af, wt, brow in (
            (out1_d, af1, w5t_sb, b5_sb),
            (out0_d, af0, w4t_sb, b4_sb),
        ):
            for nb in range(NB):
                ps = psS.tile([128, 1024], f32, tag="s")
                p2 = ps[:, 0:256]
                nc.tensor.matmul(p2, af[:, 0, ts(nb, 128)], wt[:, 0], start=True, stop=False)
                nc.tensor.matmul(p2, af[:, 1, ts(nb, 128)], wt[:, 1], start=False, stop=False)
                nc.tensor.matmul(p2, ones_t[:, 0:128], brow[:], start=False, stop=True)
                ot = opool.tile([128, 256], f32, tag="ot")
                nc.vector.tensor_scalar_max(ot[:], p2, 0.0)
                nc.sync.dma_start(out_d[ts(nb, 128), :], ot[:])

    nc.compile()
    return nc


def _prep_core_inputs(inputs):
    """Fold BN into weights, build per-core input maps."""
    x1 = np.ascontiguousarray(inputs["x1"], dtype=np.float32)
    x2 = np.ascontiguousarray(inputs["x2"], dtype=np.float32)
    Ws = np.asarray(inputs["Ws"], dtype=np.float32)
    bs = np.asarray(inputs["bs"], dtype=np.float32)
    g = np.asarray(inputs["gammas"], dtype=np.float32)
    be = np.asarray(inputs["betas"], dtype=np.float32)
    mn = np.asarray(inputs["means"], dtype=np.float32)
    vr = np.asarray(inputs["vars_"], dtype=np.float32)

    s = g / np.sqrt(vr + EPS)  # [6, C]
    Wf = Ws * s[:, :, None]  # rows scaled
    bf = s * (bs - mn) + be

    import ml_dtypes

    bfl = ml_dtypes.bfloat16
    WfT = np.ascontiguousarray(np.swapaxes(Wf, 1, 2)).astype(bfl)  # [6, C, C]
    x1t = np.ascontiguousarray(np.swapaxes(x1, 1, 2)).astype(bfl)  # [B, C, N]
    x2t = np.ascontiguousarray(np.swapaxes(x2, 1, 2)).astype(bfl)
    bfb = bf.astype(bfl)

    in_maps = []
    for core in range(NCORES):
        b, par = core // 2, core % 2
        sl = slice(par * 128, par * 128 + 128)
        in_maps.append(
            {
                "x1t": x1t[b],
                "x2t": x2t[b],
                "wq": np.ascontiguousarray(WfT[0][:, sl]),
                "wk": np.ascontiguousarray(WfT[1][:, sl]),
                "wv0": np.ascontiguousarray(WfT[2][:, sl]),
                "wv1": np.ascontiguousarray(WfT[3][:, sl]),
                "bq": np.ascontiguousarray(bf[0][sl]).reshape(128, 1),
                "bk": np.ascontiguousarray(bf[1][sl]).reshape(128, 1),
                "bv0": np.ascontiguousarray(bfb[2][sl]).reshape(1, 128),
                "bv1": np.ascontiguousarray(bfb[3][sl]).reshape(1, 128),
                "w4t": WfT[4],
                "w5t": WfT[5],
                "b4": bfb[4].reshape(1, C),
                "b5": bfb[5].reshape(1, C),
                "ones": np.ones((1, 128), bfl),
                "onescol": np.ones((128, 1), bfl),
                "masks": _masks(bfl),
            }
        )
    return in_maps


def _masks(bfl):
    m = np.zeros((2, 128), np.float32)
    m[0, :64] = 1.0
    m[1, 64:] = 1.0
    return m.astype(bfl)


def kernel(**inputs):
    from concourse import bass_utils

    if "nc" not in _CACHE:
        _CACHE["nc"] = _build_program()
    nc = _CACHE["nc"]

    in_maps = _prep_core_inputs(inputs)
    res = bass_utils.run_bass_kernel_spmd(
        nc, in_maps, core_ids=list(range(NCORES))
    )
    results = res.results
    out0 = np.stack([results[2 * b]["out0t"] for b in range(B)])
    out1 = np.stack([results[2 * b]["out1t"] for b in range(B)])
    return out0, out1
